# revision 59
# baseline (speedup 1.0000x reference)
"""Causal MHA + RoPE (B=2, T=2048, D=2048, H=16, HD=128), fp32 in/out.

Tensor-parallel over heads across 8 NeuronCores (2 heads/core), fp16
compute with fp8 (e4m3) DoubleRow QKV projections:
  - w_q/w_k/w_v column-sharded, w_o row-sharded; fp16 partial outputs
    summed in fp32 on the host.
  - Transposed on-device layout ([feature, token]); see the per-stage
    notes below.
  - QKV projections run as fp8e4 DoubleRow matmuls (2 contraction rows
    per PE cycle = 2x fp16 rate). Weights are pre-scaled by WS=64 on the
    host so their mass sits in e4m3's normal range; the 1/WS is folded
    into the RoPE cos/sin tables (q/k) and the v PSUM->SBUF copy scale.
  - fp8 quantization noise (~5% rms on q/k/v) washes out through the
    softmax for queries attending to many keys, but not for the first
    ~tens of tokens. A fp16 patch recomputes q/k/v for tokens 0..127 of
    each batch (full-precision weights, free dim 128) and overwrites
    those columns after the main RoPE pass.
  - Attention (S = kT.T @ qT, exp, esum/denominator, O += v.T @ E) and
    the out-projection stay fp16 exactly as before.
  - Out-projection PSUM->SBUF copies moved from DVE to the Pool engine
    (DVE was the #2-busy engine; Pool is nearly idle).
"""

import numpy as np

B, T, D, H = 2, 2048, 2048, 16
HD = D // H  # 128
NCORES = 8
HPC = H // NCORES  # heads per core = 2
CD = HPC * HD  # per-core head dims = 256
SCALE = 1.0 / float(np.sqrt(HD))
TB = 512  # token block (matmul free dim)
NTB = T // TB  # 4 token blocks per batch
NKT = T // 128  # 16 key tiles per batch
KO = D // 128  # 16 contraction tiles over D
KO2 = KO // 2  # 8 DoubleRow pair slices
WS = 64.0  # host-side fp8 weight pre-scale
NEG = -1.0e30


_PATCHED = False


def _apply_tile_patches():
    """This container's walrus build allows only ONE sync-wait command per
    TPB instruction (e.g. the S3_LW struct of a fused fp32 matmul rejects
    2 waits with "Too many sync wait commands"). Tile's scheduler freely
    puts several waits on one instruction. Two patches:

    1. After wait assignment, hoist all-but-one waits of every instruction
       onto injected same-engine NoOps placed just before it.
    2. The final TileContext drain aggregates all outstanding waits onto
       one SP Drain — split into a chain of single-wait drains.
    """
    global _PATCHED
    if _PATCHED:
        return
    _PATCHED = True

    import concourse.mybir as mybir
    import concourse.tile as tile
    from concourse.vector_clock import ScopedClock

    MAXW = 1

    _orig_lower = tile.TileContext._lower_ordered_insts

    def _lower_ordered_insts(self, ordered):
        nc = self.nc
        for insts in ordered.values():
            need = any(
                i.sync_info is not None and len(i.sync_info.on_wait) > MAXW
                for i in insts
            )
            if not need:
                continue
            out = []
            for inst in insts:
                si = inst.sync_info
                if si is not None and len(si.on_wait) > MAXW:
                    waits = list(si.on_wait)
                    extra = waits[MAXW:]
                    del si.on_wait[MAXW:]
                    for j in range(0, len(extra), MAXW):
                        nop = mybir.InstNoOp(
                            name=nc.get_next_instruction_name(), ins=[], outs=[]
                        )
                        nop.engine = inst.engine
                        nop.sync_info = mybir.SyncInfo(
                            on_wait=extra[j : j + MAXW], on_update=[]
                        )
                        nc.register_instruction(nop)
                        out.append(nop)
                out.append(inst)
            insts[:] = out
        return _orig_lower(self, ordered)

    def _drain_and_barrier(self, tick_clock, wait_clock):
        drain_inst = self.nc.sync.drain()
        wait_clock.add_sem_waits(
            drain_inst.ins, ScopedClock({None: tick_clock.global_clock})
        )
        si = drain_inst.ins.sync_info
        waits = list(si.on_wait) if si is not None else []
        if len(waits) > 1:
            del si.on_wait[1:]
            for w in waits[1:]:
                extra = self.nc.sync.drain()
                extra.ins.sync_info = mybir.SyncInfo(on_wait=[w], on_update=[])
        self.nc.all_engine_barrier()
        assert self.sems is not None
        popped = self.nc._tile_sem_poison_stack.pop()
        assert popped is self._sem_poison
        self.nc.clear_and_free_semaphores(list(self.sems.allocated().values()))
        self.nc.all_engine_barrier()

    tile.TileContext._lower_ordered_insts = _lower_ordered_insts
    tile.TileContext._drain_and_barrier = _drain_and_barrier


def build_bass():
    _apply_tile_patches()
    import concourse.bass as bass
    import concourse.mybir as mybir
    import concourse.tile as tile
    from concourse.masks import make_identity

    f32 = mybir.dt.float32
    f16 = mybir.dt.float16
    f8e4 = mybir.dt.float8e4
    f8e5 = mybir.dt.float8e5
    EXP = mybir.ActivationFunctionType.Exp
    COPY = mybir.ActivationFunctionType.Copy
    DR = mybir.MatmulPerfMode.DoubleRow

    nc = bass.Bass("TRN2", target_bir_lowering=False, debug=False)

    x8T = nc.dram_tensor("x8T", [B, D, T], f8e4, kind="ExternalInput").ap()
    # patch x for BOTH batches: [128p, ko, b*128+t] so one fp16 patch chain
    # (free dim 256) covers batch 0 and batch 1 together
    xfixd = nc.dram_tensor("xfixd", [128, KO, B * 128], f16, kind="ExternalInput").ap()
    wq8d = nc.dram_tensor("wq8d", [128, KO2, 2, CD], f8e4, kind="ExternalInput").ap()
    wk8d = nc.dram_tensor("wk8d", [128, KO2, 2, CD], f8e4, kind="ExternalInput").ap()
    wv8d = nc.dram_tensor("wv8d", [128, KO2, 2, CD], f8e4, kind="ExternalInput").ap()
    wqfd = nc.dram_tensor("wqfd", [D, CD], f16, kind="ExternalInput").ap()
    wkfd = nc.dram_tensor("wkfd", [D, CD], f16, kind="ExternalInput").ap()
    wvfd = nc.dram_tensor("wvfd", [D, CD], f16, kind="ExternalInput").ap()
    woT = nc.dram_tensor("woT", [CD, D], f16, kind="ExternalInput").ap()
    cosd = nc.dram_tensor("cosd", [HD, T], f16, kind="ExternalInput").ap()
    sind = nc.dram_tensor("sind", [HD, T], f16, kind="ExternalInput").ap()
    out = nc.dram_tensor("out", [B, D, T], f16, kind="ExternalOutput").ap()

    with tile.TileContext(nc) as tc:
        with (
            tc.tile_pool(name="consts", bufs=1) as cpool,
            tc.tile_pool(name="acts", bufs=1) as apool,
            tc.tile_pool(name="xs", bufs=12) as xpool,
            tc.tile_pool(name="rt", bufs=4) as rpool,
            tc.tile_pool(name="rq", bufs=4) as rqpool,
            tc.tile_pool(name="vt", bufs=2) as vtpool,
            tc.tile_pool(name="et", bufs=8) as epool,
            tc.tile_pool(name="es", bufs=4) as espool,
            tc.tile_pool(name="rc", bufs=2) as rcpool,
            tc.tile_pool(name="oc", bufs=3) as ocpool,
            tc.tile_pool(name="obp", bufs=8) as obpool,
            tc.tile_pool(name="ps", bufs=8, space="PSUM") as psp,
        ):
            # ---- persistent constants ----
            # fp8 weights, pair-major for DoubleRow: [128, ko2, pair, CD]
            wq8_sb = cpool.tile([128, KO2, 2, CD], f8e4, name="wq8_sb")
            wk8_sb = cpool.tile([128, KO2, 2, CD], f8e4, name="wk8_sb")
            wv8_sb = cpool.tile([128, KO2, 2, CD], f8e4, name="wv8_sb")
            # fp16 weights for the token-0..127 patch chains
            wqf_sb = cpool.tile([128, KO, CD], f16, name="wqf_sb")
            wkf_sb = cpool.tile([128, KO, CD], f16, name="wkf_sb")
            wvf_sb = cpool.tile([128, KO, CD], f16, name="wvf_sb")
            xfix_sb = cpool.tile([128, KO, B * 128], f16, name="xfix_sb")
            # rope tables for the patch region, tiled x2 (both batch halves)
            cosfix = cpool.tile([128, B * 128], f16, name="cosfix")
            sinfix = cpool.tile([128, B * 128], f16, name="sinfix")
            # batch-1 patch results stash (applied after b1's main rope)
            qk_stash = cpool.tile([128, 4, 128], f16, name="qk_stash")
            v_stash = cpool.tile([128, HPC, 128], f16, name="v_stash")

            def load_w8_slice(ko2, first=False):
                # wq/wv on the ACT HWDGE queue, wk on SWDGE; first slices
                # ride the SP queue — the ACT engine runs its activation
                # table load first and would delay the earliest matmuls
                qeng = nc.sync if first else nc.scalar
                qeng.dma_start(wq8_sb[:, ko2], wq8d[:, ko2])
                nc.gpsimd.dma_start(wk8_sb[:, ko2], wk8d[:, ko2])
                qeng.dma_start(wv8_sb[:, ko2], wv8d[:, ko2])

            # cross-boundary x-tile prefetches: (b, nb, quad) -> sbuf tile.
            # x quads [128, 4, TB] fp8 hold TWO DoubleRow rhs pairs each —
            # halves the SP-queue DMA trigger count (triggers, not BW, pace
            # x delivery at block boundaries).
            xt_pre = {}
            xTr = x8T.rearrange("b (kk p) t -> b p kk t", p=128)

            def load_xt(bb, nnb, q4):
                xt = xpool.tile([128, 4, TB], f8e4, name="xt", tag="xt")
                nc.sync.dma_start(
                    xt[:],
                    xTr[bb, :, 4 * q4 : 4 * q4 + 4, nnb * TB : (nnb + 1) * TB],
                )
                return xt

            # first slice split so the very first x quad queues right after
            # wq0 on the SP queue
            nc.sync.dma_start(wq8_sb[:, 0], wq8d[:, 0])
            nc.gpsimd.dma_start(wk8_sb[:, 0], wk8d[:, 0])
            xt_pre[0, 0, 0] = load_xt(0, 0, 0)
            nc.sync.dma_start(wv8_sb[:, 0], wv8d[:, 0])
            load_w8_slice(1)
            load_w8_slice(2)

            ident = cpool.tile([128, 128], f16, name="ident")
            make_identity(nc, ident)
            ones_f32 = cpool.tile([128, 128], f32, name="ones_f32")
            nc.vector.memset(ones_f32[:], 1.0)
            ones_sb = cpool.tile([128, 128], f16, name="ones_sb")
            nc.vector.tensor_copy(ones_sb[:], ones_f32[:])
            # upper-triangular (col >= partition) causal band mask: applied
            # as a DVE multiply so the Pool engine stays off the attention
            # critical path
            mask_sb = cpool.tile([128, 128], f16, name="mask_sb")
            nc.gpsimd.affine_select(
                out=mask_sb[:],
                in_=ones_sb[:],
                compare_op=mybir.AluOpType.is_ge,
                fill=0.0,
                base=0,
                pattern=[[1, 128]],
                channel_multiplier=-1,
            )
            cos_sb = cpool.tile([128, T], f16, name="cos_sb")
            sin_sb = cpool.tile([128, T], f16, name="sin_sb")
            wo_sb = cpool.tile([128, HPC, D], f16, name="wo_sb")

            # ---- per-batch activation storage (slots reused across batches) ----
            qT_sb = apool.tile([128, HPC, T], f16, name="qT_sb")
            kT_sb = apool.tile([128, HPC, T], f16, name="kT_sb")
            vh_sb = apool.tile([128, NKT, CD], f16, name="vh_sb")

            def ps_tile(nm):
                return psp.tile([128, TB], f32, name=nm, tag="ps")

            # pending projection work: list of thunks, each emits one
            # (dout, both-kk) matmul pair + copy + store
            pending = []
            # which engine takes the PSUM->SBUF ob copy at drain time
            ob_on_act = [False]
            ob_ctr = [0]

            def emit_proj_block(bb, jj, ocb, spread=False):
                tqp = slice(jj * TB, (jj + 1) * TB)

                def mk(do):
                    def thunk():
                        pp = ps_tile("pp")
                        for kk in range(HPC):
                            nc.tensor.matmul(
                                pp[:],
                                lhsT=wo_sb[:, kk, do * 128 : (do + 1) * 128],
                                rhs=ocb[:, kk, :],
                                start=(kk == 0),
                                stop=(kk == HPC - 1),
                                skip_group_check=True,
                            )
                        ob = obpool.tile([128, TB], f16, name="ob", tag="ob")
                        # PSUM->SBUF copies: ACT during QKV phases (ACT is
                        # idle there, DVE runs the ropes); alternate DVE/ACT
                        # during attention (DVE gates the late blocks)
                        if spread and do % 2 == 1:
                            nc.scalar.copy(ob[:], pp[:])
                        elif spread:
                            nc.vector.tensor_copy(ob[:], pp[:])
                        elif ob_on_act[0]:
                            nc.scalar.copy(ob[:], pp[:])
                        else:
                            nc.vector.tensor_copy(ob[:], pp[:])
                        if spread:
                            # avoid the SWDGE queue at the tail: its
                            # transfers complete late and hold up teardown
                            qeng = nc.sync if do % 2 == 0 else nc.scalar
                        else:
                            qeng = nc.sync if do % 2 == 0 else nc.gpsimd
                        qeng.dma_start(
                            out[bb, do * 128 : (do + 1) * 128, tqp], ob[:]
                        )

                    return thunk

                for do in range(D // 128):
                    pending.append(mk(do))

            def drain_pending(k):
                for _ in range(min(k, len(pending))):
                    pending.pop(0)()

            for b in range(B):
                # attention helpers (defined before the QKV loop so the last
                # QKV block can pre-emit the first attention tiles)
                def s_mm(j4, h, i):
                    s = ps_tile("s_ps")
                    p = i - 4 * j4
                    # matmuls narrower than 256 free run at 1/4 rate, so
                    # pad the p=3 diagonal tile to 256 (extra cols are
                    # masked later)
                    c0 = min(128 * p, TB - 256) if p > 0 else 0
                    nc.tensor.matmul(
                        s[:, c0:],
                        lhsT=kT_sb[:, h, i * 128 : (i + 1) * 128],
                        rhs=qT_sb[:, h, j4 * TB + c0 : (j4 + 1) * TB],
                        start=True,
                        stop=True,
                        skip_group_check=True,
                    )
                    return s

                def exp_tile(j4, h, i, s):
                    e_sb = epool.tile([128, TB], f16, name="e_sb", tag="e")
                    p = i - 4 * j4
                    if p < 0:
                        nc.scalar.activation(e_sb[:], s[:], EXP, scale=SCALE)
                    else:
                        # diagonal tile: cols < 128p never read downstream
                        # (o/esum start at min(c0, TB-256)), the 128-wide
                        # band [128p, 128p+128) is triangular, cols >=
                        # 128p+128 fully valid
                        c0 = 128 * p
                        mc0 = min(c0, TB - 256)
                        nc.scalar.activation(
                            e_sb[:, c0:], s[:, c0:], EXP, scale=SCALE
                        )
                        nc.vector.tensor_mul(
                            e_sb[:, c0 : c0 + 128],
                            e_sb[:, c0 : c0 + 128],
                            mask_sb[:],
                        )
                        if mc0 < c0:
                            nc.vector.memset(e_sb[:, mc0:c0], 0)
                    return e_sb

                # fp16 patch chains: recompute q/k/v for tokens 0..127 at
                # full precision and overwrite the fp8-path values (fp8
                # noise doesn't wash out for early tokens' softmax). One
                # chain covers BOTH batches (free dim 256); batch 1 results
                # stash until after b1's main rope, then copy in.
                def emit_patch(w, m):
                    wf_sb = {"q": wqf_sb, "k": wkf_sb, "v": wvf_sb}[w]
                    pp = psp.tile([128, B * 128], f32, name="pfix", tag="ps")
                    for ko in range(KO):
                        nc.tensor.matmul(
                            pp[:],
                            lhsT=wf_sb[:, ko, m * 128 : (m + 1) * 128],
                            rhs=xfix_sb[:, ko, :],
                            start=(ko == 0),
                            stop=(ko == KO - 1),
                        )
                    if w == "v":
                        vtmp = rpool.tile([128, TB], f16, name="rtmp", tag="rtmp")
                        nc.scalar.activation(
                            vtmp[:, 0:256], pp[:], COPY, scale=1.0 / WS
                        )
                        vt_ps = psp.tile([128, 2, 128], f16, name="vt_ps", tag="ps")
                        for half in range(2):
                            nc.tensor.matmul(
                                vt_ps[:, half, :],
                                lhsT=vtmp[:, half * 128 : half * 128 + 128],
                                rhs=ident[:],
                                is_transpose=True,
                                start=(half == 0),
                                stop=(half == 1),
                                skip_group_check=True,
                            )
                        nc.scalar.copy(
                            vh_sb[:, 0, m * 128 : (m + 1) * 128], vt_ps[:, 0, :]
                        )
                        nc.scalar.copy(v_stash[:, m, :], vt_ps[:, 1, :])
                    else:
                        rr = rpool.tile([128, TB], f16, name="rtmp", tag="rtmp")
                        tmp = rpool.tile([128, TB], f16, name="rtmp", tag="rtmp")
                        nc.vector.tensor_mul(rr[:, 0:256], pp[:], cosfix[:])
                        nc.vector.tensor_mul(
                            tmp[0:64, 0:256], pp[64:128, :], sinfix[0:64, :]
                        )
                        nc.vector.tensor_mul(
                            tmp[64:128, 0:256], pp[0:64, :], sinfix[64:128, :]
                        )
                        nc.vector.tensor_add(rr[:, 0:256], rr[:, 0:256], tmp[:, 0:256])
                        dst = qT_sb if w == "q" else kT_sb
                        nc.vector.tensor_copy(dst[:, m, 0:128], rr[:, 0:128])
                        si = (0 if w == "q" else 2) + m
                        nc.vector.tensor_copy(qk_stash[:, si, :], rr[:, 128:256])

                def apply_stash(w, m):
                    if w == "v":
                        nc.scalar.copy(
                            vh_sb[:, 0, m * 128 : (m + 1) * 128], v_stash[:, m, :]
                        )
                    else:
                        si = (0 if w == "q" else 2) + m
                        dst = qT_sb if w == "q" else kT_sb
                        nc.vector.tensor_copy(dst[:, m, 0:128], qk_stash[:, si, :])

                patch_fn = emit_patch if b == 0 else apply_stash
                patch_queue = [
                    ("q", 0), ("q", 1), ("k", 0), ("k", 1), ("v", 0), ("v", 1)
                ]

                # carried across blocks: S psums / exp tiles pre-emitted at
                # the previous block's tail so the next block's PE/ACT work
                # is already queued while the divide chain drains
                s_pend = {}
                e_pend = {}
                # ============ QKV projections (+RoPE, v transpose) ============
                for nb in range(NTB):
                    tsl = slice(nb * TB, (nb + 1) * TB)
                    psums = {}

                    def v_par(m):
                        vtt = vtpool.tile([128, TB], f16, name="vtt", tag="vtt")
                        nc.scalar.activation(
                            vtt[:], psums["v", m][:], COPY, scale=1.0 / WS
                        )
                        vt_ps = psp.tile([128, 4, 128], f16, name="vt_ps", tag="ps")
                        for tti in range(4):
                            nc.tensor.matmul(
                                vt_ps[:, tti, :],
                                lhsT=vtt[:, tti * 128 : (tti + 1) * 128],
                                rhs=ident[:],
                                is_transpose=True,
                                start=(tti == 0),
                                stop=(tti == 3),
                                skip_group_check=True,
                            )
                        nc.scalar.copy(
                            vh_sb[:, nb * 4 : nb * 4 + 4, m * 128 : (m + 1) * 128],
                            vt_ps[:],
                        )

                    def rope_par(w, dst, m):
                        ps = psums[w, m]
                        tmp = rpool.tile([128, TB], f16, name="rtmp", tag="rtmp")
                        d = dst[:, m, tsl]
                        nc.vector.tensor_mul(d, ps[:], cos_sb[:, tsl])
                        nc.vector.tensor_mul(
                            tmp[0:64, :], ps[64:128, :], sin_sb[0:64, tsl]
                        )
                        nc.vector.tensor_mul(
                            tmp[64:128, :], ps[0:64, :], sin_sb[64:128, tsl]
                        )
                        nc.vector.tensor_add(d, d, tmp[:])

                    if b == 0 and nb == 0:
                        # first block: weights are still streaming in, so run
                        # the ko2-interleaved order that matches their arrival
                        for w in ("q", "k", "v"):
                            for m in range(HPC):
                                psums[w, m] = ps_tile(f"ps_{w}{m}")
                        xq = None
                        for ko2 in range(KO2):
                            if ko2 % 2 == 0:
                                q4 = ko2 // 2
                                xq = xt_pre.pop((b, nb, q4), None)
                                if xq is None:
                                    xq = load_xt(b, nb, q4)
                            rhs = xq[:, 2 * (ko2 % 2) : 2 * (ko2 % 2) + 2, :]
                            for w, w_sb in (
                                ("q", wq8_sb), ("k", wk8_sb), ("v", wv8_sb)
                            ):
                                for m in range(HPC):
                                    nc.tensor.matmul(
                                        psums[w, m][:],
                                        lhsT=w_sb[:, ko2, :, m * 128 : (m + 1) * 128],
                                        rhs=rhs,
                                        start=(ko2 == 0),
                                        stop=(ko2 == KO2 - 1),
                                        perf_mode=DR,
                                    )
                            if ko2 in (0, 1, 2):
                                load_w8_slice(ko2 + 3)
                            elif ko2 == 3:
                                load_w8_slice(6)
                                load_w8_slice(7)
                            if ko2 in (0, 2, 4):
                                # JIT the next x quad (needed at step ko2+2)
                                q4n = ko2 // 2 + 1
                                if (b, nb, q4n) not in xt_pre:
                                    xt_pre[b, nb, q4n] = load_xt(b, nb, q4n)
                            if ko2 == 4:
                                nc.gpsimd.dma_start(cos_sb[:], cosd)
                            elif ko2 == 5:
                                nc.gpsimd.dma_start(sin_sb[:], sind)
                            elif ko2 == 7:
                                # first fp16 patch weight; the rest (wkf, wvf,
                                # wo) are staggered over blocks 1-2 so their
                                # 4MB doesn't crowd out the x-quad stream
                                nc.scalar.dma_start(
                                    wqf_sb[:],
                                    wqfd.rearrange("(kk p) n -> p kk n", p=128),
                                )
                            if ko2 >= 4:
                                pk = ko2 - 4
                                xt_pre[b, nb + 1, pk] = load_xt(b, nb + 1, pk)
                        # q first: attention needs q of this block immediately
                        v_par(0)
                        rope_par("q", qT_sb, 0)
                        rope_par("q", qT_sb, 1)
                        v_par(1)
                        rope_par("k", kT_sb, 0)
                        rope_par("k", kT_sb, 1)
                    else:
                        # steady state: one output at a time over all 8 pair
                        # slices (x pairs prefetched during the previous
                        # block), post-processing each output while the next
                        # accumulates.
                        xts = []
                        for q4 in range(KO2 // 2):
                            xq = xt_pre.pop((b, nb, q4), None)
                            if xq is None:
                                xq = load_xt(b, nb, q4)
                            xts.append(xq)
                        if b == 0 and nb == 1:
                            # patch x (both batches) + tiled rope tables
                            nc.sync.dma_start(xfix_sb[:], xfixd)
                            for half in range(B):
                                hs = slice(half * 128, (half + 1) * 128)
                                nc.vector.tensor_copy(cosfix[:, hs], cos_sb[:, 0:128])
                                nc.vector.tensor_copy(sinfix[:, hs], sin_sb[:, 0:128])
                        if b == 0 and nb == 2:
                            nc.scalar.dma_start(
                                wvf_sb[:],
                                wvfd.rearrange("(kk p) n -> p kk n", p=128),
                            )

                        def mm_pass(w, m):
                            w_sb = {"q": wq8_sb, "k": wk8_sb, "v": wv8_sb}[w]
                            ps = ps_tile(f"ps_{w}{m}")
                            psums[w, m] = ps
                            for ko2 in range(KO2):
                                nc.tensor.matmul(
                                    ps[:],
                                    lhsT=w_sb[:, ko2, :, m * 128 : (m + 1) * 128],
                                    rhs=xts[ko2 // 2][
                                        :, 2 * (ko2 % 2) : 2 * (ko2 % 2) + 2, :
                                    ],
                                    start=(ko2 == 0),
                                    stop=(ko2 == KO2 - 1),
                                    perf_mode=DR,
                                )

                        def v_copy(m):
                            vtt = vtpool.tile([128, TB], f16, name="vtt", tag="vtt")
                            nc.scalar.activation(
                                vtt[:], psums["v", m][:], COPY, scale=1.0 / WS
                            )
                            return vtt

                        def v_tr(m, vtt):
                            # 4 transposes accumulate into ONE psum bank
                            # (start only on the first — a start marks the
                            # whole 2KB bank pending-zero), then a single
                            # strided ACT copy casts to fp8 vh slots
                            vt_ps = psp.tile([128, 4, 128], f16, name="vt_ps", tag="ps")
                            for tti in range(4):
                                nc.tensor.matmul(
                                    vt_ps[:, tti, :],
                                    lhsT=vtt[:, tti * 128 : (tti + 1) * 128],
                                    rhs=ident[:],
                                    is_transpose=True,
                                    start=(tti == 0),
                                    stop=(tti == 3),
                                    skip_group_check=True,
                                )
                            nc.scalar.copy(
                                vh_sb[:, nb * 4 : nb * 4 + 4, m * 128 : (m + 1) * 128],
                                vt_ps[:],
                            )

                        def pf(oi):
                            if nb + 1 < NTB and oi < 4:
                                if (b, nb + 1, oi) not in xt_pre:
                                    xt_pre[b, nb + 1, oi] = load_xt(b, nb + 1, oi)
                                # reach one block further for the first two
                                # quads: absorbs DMA bursts (weights/stores)
                                # without stalling the next block's matmuls
                                if oi < 2 and nb + 2 < NTB and (
                                    (b, nb + 2, oi) not in xt_pre
                                ):
                                    xt_pre[b, nb + 2, oi] = load_xt(b, nb + 2, oi)

                        def patch(k=1):
                            if nb >= 2:
                                for _ in range(min(k, len(patch_queue))):
                                    w, m = patch_queue.pop(0)
                                    patch_fn(w, m)

                        # each v pass only issues its ACT copy; the PE
                        # transposes run a pass later, once the copy has had
                        # a full matmul pass of latency cover. k1 goes last:
                        # its rope is DVE-only, so the block tail never
                        # blocks the PE.
                        ob_on_act[0] = True
                        mm_pass("q", 0)
                        if len(pending) > 4:
                            drain_pending(min(2, len(pending) - 4))
                        rope_par("q", qT_sb, 0)
                        pf(0)
                        patch()
                        mm_pass("q", 1)
                        if len(pending) > 4:
                            drain_pending(min(2, len(pending) - 4))
                        rope_par("q", qT_sb, 1)
                        pf(1)
                        patch()
                        mm_pass("k", 0)
                        if len(pending) > 4:
                            drain_pending(min(2, len(pending) - 4))
                        rope_par("k", kT_sb, 0)
                        pf(2)
                        patch()
                        mm_pass("v", 0)
                        vtt0 = v_copy(0)
                        if len(pending) > 4:
                            drain_pending(min(2, len(pending) - 4))
                        pf(3)
                        mm_pass("v", 1)
                        vtt1 = v_copy(1)
                        v_tr(0, vtt0)
                        if len(pending) > 4:
                            drain_pending(min(2, len(pending) - 4))
                        mm_pass("k", 1)
                        v_tr(1, vtt1)
                        rope_par("k", kT_sb, 1)
                        if len(pending) > 4:
                            drain_pending(min(2, len(pending) - 4))
                        if b == 0 and nb == 1:
                            nc.gpsimd.dma_start(
                                wkf_sb[:],
                                wkfd.rearrange("(kk p) n -> p kk n", p=128),
                            )
                        elif b == 0 and nb == 2:
                            nc.gpsimd.dma_start(
                                wo_sb[:],
                                woT.rearrange("(kk p) n -> p kk n", p=128),
                            )

                # any patches not yet emitted (shouldn't happen) run now
                while patch_queue:
                    w, m = patch_queue.pop(0)
                    patch_fn(w, m)

                # ============ attention (staggered heads) + spread proj ============
                ob_on_act[0] = False
                for j4 in range(NTB):
                    tq = slice(j4 * TB, (j4 + 1) * TB)
                    n_tk = 4 * (j4 + 1)
                    ocb = ocpool.tile([128, HPC, TB], f16, name="ocb", tag="ocb")
                    o_ps = [ps_tile(f"o_ps{h}") for h in range(HPC)]
                    # softmax denominators: E tiles summed on DVE (fp16),
                    # finished by one small ones-matmul per head — keeps
                    # ~30us of denominator matmuls off the PE
                    # two accumulators per head (even/odd key tiles): halves
                    # the serial DVE add-chain latency before the divide;
                    # the den ones-matmul sums both into one PSUM for free
                    esum = [
                        [
                            espool.tile([128, TB], f16, name=f"esum{h}{p}", tag="es")
                            for p in range(2)
                        ]
                        for h in range(HPC)
                    ]

                    def o_den_mm(h, i, e_sb):
                        p = i - 4 * j4
                        c0 = min(128 * p, TB - 256) if p > 0 else 0
                        nc.tensor.matmul(
                            o_ps[h][:, c0:],
                            lhsT=vh_sb[:, i, h * 128 : (h + 1) * 128],
                            rhs=e_sb[:, c0:],
                            start=(i == 0),
                            stop=(i == n_tk - 1),
                            skip_group_check=True,
                        )
                        # even chain on DVE, odd chain on Pool (SBUF-only op,
                        # not latency-critical until the block's divide)
                        eng = nc.vector if i % 2 == 0 else nc.gpsimd
                        es = esum[h][i % 2]
                        if i < 2:
                            if c0 > 0:
                                eng.memset(es[:, 0:c0], 0)
                            eng.tensor_copy(es[:, c0:], e_sb[:, c0:])
                        else:
                            eng.tensor_add(
                                es[:, c0:], es[:, c0:], e_sb[:, c0:]
                            )

                    def emit_div(h):
                        den = ps_tile("den")
                        for p in range(2):
                            nc.tensor.matmul(
                                den[:],
                                lhsT=ones_sb[:],
                                rhs=esum[h][p][:],
                                start=(p == 0),
                                stop=(p == 1),
                                skip_group_check=True,
                            )
                        lnd = rcpool.tile([128, TB], f32, name="lnd", tag="lnd")
                        nc.scalar.activation(
                            lnd[:], den[:], mybir.ActivationFunctionType.Ln
                        )
                        recip = rcpool.tile([128, TB], f32, name="recip", tag="rcp")
                        nc.scalar.activation(recip[:], lnd[:], EXP, scale=-1.0)
                        nc.vector.tensor_mul(ocb[:, h, :], o_ps[h][:], recip[:])

                    if j4 == 0:
                        # all of head 0 first: every h1 S tile here needs
                        # this batch's k1, whose rope is the QKV-phase tail
                        # on DVE — h0's full pass covers that latency
                        for h in range(HPC):
                            if (h, 0) not in s_pend and (h, 0) not in e_pend:
                                s_pend[h, 0] = s_mm(j4, h, 0)
                            for i in range(n_tk):
                                if i + 1 < n_tk and (h, i + 1) not in s_pend:
                                    s_pend[h, i + 1] = s_mm(j4, h, i + 1)
                                elif i + 1 == n_tk and h + 1 < HPC:
                                    s_pend[h + 1, 0] = s_mm(j4, h + 1, 0)
                                e = e_pend.pop((h, i), None)
                                if e is None:
                                    e = exp_tile(j4, h, i, s_pend.pop((h, i)))
                                o_den_mm(h, i, e)
                            if h == 0:
                                emit_div(0)
                            else:
                                ns00 = s_mm(j4 + 1, 0, 0)
                                s_pend[1, 0] = s_mm(j4 + 1, 1, 0)
                                s_pend[0, 1] = s_mm(j4 + 1, 0, 1)
                                e_pend[0, 0] = exp_tile(j4 + 1, 0, 0, ns00)
                            drain_pending(2)
                        emit_div(1)
                    else:
                        for i in range(n_tk):
                            if (1, i) not in s_pend:
                                s_pend[1, i] = s_mm(j4, 1, i)
                            if i + 1 < n_tk and (0, i + 1) not in s_pend:
                                s_pend[0, i + 1] = s_mm(j4, 0, i + 1)
                            e0 = e_pend.pop((0, i), None)
                            if e0 is None:
                                e0 = exp_tile(j4, 0, i, s_pend.pop((0, i)))
                            o_den_mm(0, i, e0)
                            if i == n_tk - 1:
                                # head 0 finished: divide now so its o/den
                                # psum banks free before the next block
                                emit_div(0)
                            e1 = exp_tile(j4, 1, i, s_pend.pop((1, i)))
                            o_den_mm(1, i, e1)
                            if i == n_tk - 1 and j4 + 1 < NTB:
                                # pre-emit the next block's first S matmuls
                                # and exp ahead of this block's divide chain,
                                # so neither the PE nor ACT queue drains dry
                                # at the block boundary
                                ns00 = s_mm(j4 + 1, 0, 0)
                                s_pend[1, 0] = s_mm(j4 + 1, 1, 0)
                                s_pend[0, 1] = s_mm(j4 + 1, 0, 1)
                                e_pend[0, 0] = exp_tile(j4 + 1, 0, 0, ns00)
                            # drain the out-projection backlog, keeping >=4
                            # thunks in reserve for the boundary
                            if 1 <= i < n_tk - 2 and len(pending) > 4:
                                drain_pending(min(3, len(pending) - 4))
                        emit_div(1)
                    drain_pending(4)
                    emit_proj_block(
                        b, j4, ocb, spread=(b == B - 1 and j4 == NTB - 1)
                    )
                if b + 1 < B:
                    # prefetch the next batch's whole first x block across
                    # the QKV-phase boundary
                    for q4 in range(KO2 // 2):
                        xt_pre[b + 1, 0, q4] = load_xt(b + 1, 0, q4)
            drain_pending(len(pending))
    return nc


def prepare_inputs(x, rope_freqs, w_q, w_k, w_v, w_o):
    """Host-side sharding/layout prep. Returns per-core input maps."""
    import ml_dtypes

    E4 = ml_dtypes.float8_e4m3

    x = np.asarray(x, dtype=np.float32)
    rope_freqs = np.asarray(rope_freqs, dtype=np.float32)
    w_q = np.asarray(w_q, dtype=np.float32)
    w_k = np.asarray(w_k, dtype=np.float32)
    w_v = np.asarray(w_v, dtype=np.float32)
    w_o = np.asarray(w_o, dtype=np.float32)

    xT = np.ascontiguousarray(x.transpose(0, 2, 1))  # [B, D, T] fp32
    x8T = xT.astype(E4)
    # patch x: tokens 0..127 of BOTH batches, laid out [128p, ko, b*128+t]
    xfix = np.ascontiguousarray(
        xT[:, :, 0:128].reshape(B, KO, 128, 128).transpose(2, 1, 0, 3)
        .reshape(128, KO, B * 128)
        .astype(np.float16)
    )

    # permute q/k weight rows within each head: even HD idx -> rows 0..63,
    # odd -> rows 64..127 (so RoPE pairing becomes a half swap)
    perm = np.concatenate([np.arange(0, HD, 2), np.arange(1, HD, 2)])
    rows = (np.arange(D).reshape(H, HD)[:, perm]).reshape(D)
    w_qp = w_q[rows] * WS
    w_kp = w_k[rows] * WS
    w_vs = w_v * WS

    cos = rope_freqs[..., 0].T / WS  # [64, T]
    sin = rope_freqs[..., 1].T / WS
    cos_sb = np.ascontiguousarray(np.concatenate([cos, cos], axis=0))  # [128, T]
    sin_sb = np.ascontiguousarray(np.concatenate([-sin, sin], axis=0))

    def pack8(wT):  # wT [D, CD] fp32 -> [128, KO2, 2, CD] e4m3
        return np.ascontiguousarray(
            wT.reshape(KO2, 2, 128, CD).transpose(2, 0, 1, 3).astype(E4)
        )

    in_maps = []
    for cidx in range(NCORES):
        sl = slice(cidx * CD, (cidx + 1) * CD)
        wqT = np.ascontiguousarray(w_qp[sl].T)  # [D, CD] fp32, x WS
        wkT = np.ascontiguousarray(w_kp[sl].T)
        wvT = np.ascontiguousarray(w_vs[sl].T)
        in_maps.append(
            {
                "x8T": x8T,
                "xfixd": xfix,
                "wq8d": pack8(wqT),
                "wk8d": pack8(wkT),
                "wv8d": pack8(wvT),
                "wqfd": wqT.astype(np.float16),
                "wkfd": wkT.astype(np.float16),
                "wvfd": wvT.astype(np.float16),
                "woT": np.ascontiguousarray(w_o[:, sl].T.astype(np.float16)),
                "cosd": cos_sb.astype(np.float16),
                "sind": sin_sb.astype(np.float16),
            }
        )
    return in_maps


def run(in_maps, trace=False, tmpdir=None):
    from concourse.bass_utils import run_bass_kernel_spmd

    nc = build_bass()
    res = run_bass_kernel_spmd(
        nc,
        in_maps,
        core_ids=list(range(NCORES)),
        trace=trace,
        tmpdir=tmpdir,
    )
    total = np.zeros((B, D, T), dtype=np.float32)
    for cres in res.results:
        total += cres["out"].astype(np.float32)
    final = np.ascontiguousarray(total.transpose(0, 2, 1))  # [B, T, D]
    return final, res


def kernel(x, rope_freqs, w_q, w_k, w_v, w_o):
    in_maps = prepare_inputs(x, rope_freqs, w_q, w_k, w_v, w_o)
    final, _ = run(in_maps, trace=False)
    return final


# revision 63
# speedup vs baseline: 1.1271x; 1.1271x over previous
"""Causal MHA + RoPE (B=2, T=2048, D=2048, H=16, HD=128), fp32 in/out.

Tensor-parallel over heads across 8 NeuronCores (2 heads/core), fp16
compute with fp8 (e4m3) DoubleRow QKV projections:
  - w_q/w_k/w_v column-sharded, w_o row-sharded; fp16 partial outputs
    summed in fp32 on the host.
  - Transposed on-device layout ([feature, token]); see the per-stage
    notes below.
  - QKV projections run as fp8e4 DoubleRow matmuls (2 contraction rows
    per PE cycle = 2x fp16 rate). Weights are pre-scaled by WS=64 on the
    host so their mass sits in e4m3's normal range; the 1/WS is folded
    into the RoPE cos/sin tables (q/k) and the v PSUM->SBUF copy scale.
  - fp8 quantization noise (~5% rms on q/k/v) washes out through the
    softmax for queries attending to many keys, but not for the first
    ~tens of tokens. A fp16 patch recomputes q/k/v for tokens 0..127 of
    each batch (full-precision weights, free dim 128) and overwrites
    those columns after the main RoPE pass.
  - Attention (S = kT.T @ qT, exp, esum/denominator, O += v.T @ E) and
    the out-projection stay fp16 exactly as before.
  - Out-projection PSUM->SBUF copies moved from DVE to the Pool engine
    (DVE was the #2-busy engine; Pool is nearly idle).
"""

import numpy as np

B, T, D, H = 2, 2048, 2048, 16
HD = D // H  # 128
NCORES = 8
HPC = H // NCORES  # heads per core = 2
CD = HPC * HD  # per-core head dims = 256
SCALE = 1.0 / float(np.sqrt(HD))
TB = 512  # token block (matmul free dim)
NTB = T // TB  # 4 token blocks per batch
NKT = T // 128  # 16 key tiles per batch
KO = D // 128  # 16 contraction tiles over D
KO2 = KO // 2  # 8 DoubleRow pair slices
WS = 64.0  # host-side fp8 weight pre-scale
NEG = -1.0e30


_PATCHED = False


def _apply_tile_patches():
    """This container's walrus build allows only ONE sync-wait command per
    TPB instruction (e.g. the S3_LW struct of a fused fp32 matmul rejects
    2 waits with "Too many sync wait commands"). Tile's scheduler freely
    puts several waits on one instruction. Two patches:

    1. After wait assignment, hoist all-but-one waits of every instruction
       onto injected same-engine NoOps placed just before it.
    2. The final TileContext drain aggregates all outstanding waits onto
       one SP Drain — split into a chain of single-wait drains.
    """
    global _PATCHED
    if _PATCHED:
        return
    _PATCHED = True

    import concourse.mybir as mybir
    import concourse.tile as tile
    from concourse.vector_clock import ScopedClock

    MAXW = 1

    _orig_lower = tile.TileContext._lower_ordered_insts

    def _lower_ordered_insts(self, ordered):
        nc = self.nc
        for insts in ordered.values():
            need = any(
                i.sync_info is not None and len(i.sync_info.on_wait) > MAXW
                for i in insts
            )
            if not need:
                continue
            out = []
            for inst in insts:
                si = inst.sync_info
                if si is not None and len(si.on_wait) > MAXW:
                    waits = list(si.on_wait)
                    extra = waits[MAXW:]
                    del si.on_wait[MAXW:]
                    for j in range(0, len(extra), MAXW):
                        nop = mybir.InstNoOp(
                            name=nc.get_next_instruction_name(), ins=[], outs=[]
                        )
                        nop.engine = inst.engine
                        nop.sync_info = mybir.SyncInfo(
                            on_wait=extra[j : j + MAXW], on_update=[]
                        )
                        nc.register_instruction(nop)
                        out.append(nop)
                out.append(inst)
            insts[:] = out
        return _orig_lower(self, ordered)

    def _drain_and_barrier(self, tick_clock, wait_clock):
        drain_inst = self.nc.sync.drain()
        wait_clock.add_sem_waits(
            drain_inst.ins, ScopedClock({None: tick_clock.global_clock})
        )
        si = drain_inst.ins.sync_info
        waits = list(si.on_wait) if si is not None else []
        if len(waits) > 1:
            del si.on_wait[1:]
            for w in waits[1:]:
                extra = self.nc.sync.drain()
                extra.ins.sync_info = mybir.SyncInfo(on_wait=[w], on_update=[])
        self.nc.all_engine_barrier()
        assert self.sems is not None
        popped = self.nc._tile_sem_poison_stack.pop()
        assert popped is self._sem_poison
        self.nc.clear_and_free_semaphores(list(self.sems.allocated().values()))
        self.nc.all_engine_barrier()

    tile.TileContext._lower_ordered_insts = _lower_ordered_insts
    tile.TileContext._drain_and_barrier = _drain_and_barrier


def build_bass():
    _apply_tile_patches()
    import concourse.bass as bass
    import concourse.mybir as mybir
    import concourse.tile as tile
    from concourse.masks import make_identity

    f32 = mybir.dt.float32
    f16 = mybir.dt.float16
    f8e4 = mybir.dt.float8e4
    f8e5 = mybir.dt.float8e5
    EXP = mybir.ActivationFunctionType.Exp
    COPY = mybir.ActivationFunctionType.Copy
    DR = mybir.MatmulPerfMode.DoubleRow

    nc = bass.Bass("TRN2", target_bir_lowering=False, debug=False)

    x8T = nc.dram_tensor("x8T", [B, D, T], f8e4, kind="ExternalInput").ap()
    # patch x for BOTH batches: [128p, ko, b*128+t] so one fp16 patch chain
    # (free dim 256) covers batch 0 and batch 1 together
    xfixd = nc.dram_tensor("xfixd", [128, KO, B * 128], f16, kind="ExternalInput").ap()
    wq8d = nc.dram_tensor("wq8d", [128, KO2, 2, CD], f8e4, kind="ExternalInput").ap()
    wk8d = nc.dram_tensor("wk8d", [128, KO2, 2, CD], f8e4, kind="ExternalInput").ap()
    wv8d = nc.dram_tensor("wv8d", [128, KO2, 2, CD], f8e4, kind="ExternalInput").ap()
    wqfd = nc.dram_tensor("wqfd", [D, CD], f16, kind="ExternalInput").ap()
    wkfd = nc.dram_tensor("wkfd", [D, CD], f16, kind="ExternalInput").ap()
    wvfd = nc.dram_tensor("wvfd", [D, CD], f16, kind="ExternalInput").ap()
    woT = nc.dram_tensor("woT", [CD, D], f16, kind="ExternalInput").ap()
    cosd = nc.dram_tensor("cosd", [HD, T], f16, kind="ExternalInput").ap()
    sind = nc.dram_tensor("sind", [HD, T], f16, kind="ExternalInput").ap()
    out = nc.dram_tensor("out", [B, D, T], f16, kind="ExternalOutput").ap()

    with tile.TileContext(nc) as tc:
        with (
            tc.tile_pool(name="consts", bufs=1) as cpool,
            tc.tile_pool(name="acts", bufs=1) as apool,
            tc.tile_pool(name="xs", bufs=12) as xpool,
            tc.tile_pool(name="rt", bufs=4) as rpool,
            tc.tile_pool(name="rq", bufs=4) as rqpool,
            tc.tile_pool(name="vt", bufs=2) as vtpool,
            tc.tile_pool(name="et", bufs=8) as epool,
            tc.tile_pool(name="es", bufs=2) as espool,
            tc.tile_pool(name="rc", bufs=2) as rcpool,
            tc.tile_pool(name="oc", bufs=3) as ocpool,
            tc.tile_pool(name="obp", bufs=8) as obpool,
            tc.tile_pool(name="ps", bufs=8, space="PSUM") as psp,
        ):
            # ---- persistent constants ----
            # fp8 weights, pair-major for DoubleRow: [128, ko2, pair, CD]
            wq8_sb = cpool.tile([128, KO2, 2, CD], f8e4, name="wq8_sb")
            wk8_sb = cpool.tile([128, KO2, 2, CD], f8e4, name="wk8_sb")
            wv8_sb = cpool.tile([128, KO2, 2, CD], f8e4, name="wv8_sb")
            # fp16 weights for the token-0..127 patch chains
            wqf_sb = cpool.tile([128, KO, CD], f16, name="wqf_sb")
            wkf_sb = cpool.tile([128, KO, CD], f16, name="wkf_sb")
            wvf_sb = cpool.tile([128, KO, CD], f16, name="wvf_sb")
            xfix_sb = cpool.tile([128, KO, B * 128], f16, name="xfix_sb")
            # rope tables for the patch region, tiled x2 (both batch halves)
            cosfix = cpool.tile([128, B * 128], f16, name="cosfix")
            sinfix = cpool.tile([128, B * 128], f16, name="sinfix")
            # batch-1 patch results stash (applied after b1's main rope)
            qk_stash = cpool.tile([128, 4, 128], f16, name="qk_stash")
            v_stash = cpool.tile([128, HPC, 128], f16, name="v_stash")

            def load_w8_slice(ko2, first=False):
                # wq/wv on the ACT HWDGE queue, wk on SWDGE; first slices
                # ride the SP queue — the ACT engine runs its activation
                # table load first and would delay the earliest matmuls
                qeng = nc.sync if first else nc.scalar
                qeng.dma_start(wq8_sb[:, ko2], wq8d[:, ko2])
                nc.gpsimd.dma_start(wk8_sb[:, ko2], wk8d[:, ko2])
                qeng.dma_start(wv8_sb[:, ko2], wv8d[:, ko2])

            # cross-boundary x-tile prefetches: (b, nb, quad) -> sbuf tile.
            # x quads [128, 4, TB] fp8 hold TWO DoubleRow rhs pairs each —
            # halves the SP-queue DMA trigger count (triggers, not BW, pace
            # x delivery at block boundaries).
            xt_pre = {}
            xTr = x8T.rearrange("b (kk p) t -> b p kk t", p=128)

            def load_xt(bb, nnb, q4):
                xt = xpool.tile([128, 4, TB], f8e4, name="xt", tag="xt")
                nc.sync.dma_start(
                    xt[:],
                    xTr[bb, :, 4 * q4 : 4 * q4 + 4, nnb * TB : (nnb + 1) * TB],
                )
                return xt

            # first slice split so the very first x quad queues right after
            # wq0 on the SP queue
            nc.sync.dma_start(wq8_sb[:, 0], wq8d[:, 0])
            nc.gpsimd.dma_start(wk8_sb[:, 0], wk8d[:, 0])
            xt_pre[0, 0, 0] = load_xt(0, 0, 0)
            nc.sync.dma_start(wv8_sb[:, 0], wv8d[:, 0])
            load_w8_slice(1)
            load_w8_slice(2)

            ident = cpool.tile([128, 128], f16, name="ident")
            make_identity(nc, ident)
            ones_f32 = cpool.tile([128, 128], f32, name="ones_f32")
            nc.vector.memset(ones_f32[:], 1.0)
            ones_sb = cpool.tile([128, 128], f16, name="ones_sb")
            nc.vector.tensor_copy(ones_sb[:], ones_f32[:])
            # upper-triangular (col >= partition) causal band mask: applied
            # as a DVE multiply so the Pool engine stays off the attention
            # critical path
            mask_sb = cpool.tile([128, 128], f16, name="mask_sb")
            nc.gpsimd.affine_select(
                out=mask_sb[:],
                in_=ones_sb[:],
                compare_op=mybir.AluOpType.is_ge,
                fill=0.0,
                base=0,
                pattern=[[1, 128]],
                channel_multiplier=-1,
            )
            cos_sb = cpool.tile([128, T], f16, name="cos_sb")
            sin_sb = cpool.tile([128, T], f16, name="sin_sb")
            wo_sb = cpool.tile([128, HPC, D], f16, name="wo_sb")

            # ---- per-batch activation storage (slots reused across batches) ----
            qT_sb = apool.tile([128, HPC, T], f16, name="qT_sb")
            kT_sb = apool.tile([128, HPC, T], f16, name="kT_sb")
            vh_sb = apool.tile([128, NKT, CD], f16, name="vh_sb")

            def ps_tile(nm):
                return psp.tile([128, TB], f32, name=nm, tag="ps")

            # pending projection work: list of thunks, each emits one
            # (dout, both-kk) matmul pair + copy + store
            pending = []
            # which engine takes the PSUM->SBUF ob copy at drain time
            ob_on_act = [False]
            ob_ctr = [0]

            def emit_proj_block(bb, jj, ocb, spread=False):
                tqp = slice(jj * TB, (jj + 1) * TB)

                def mk(do):
                    def thunk():
                        pp = ps_tile("pp")
                        for kk in range(HPC):
                            nc.tensor.matmul(
                                pp[:],
                                lhsT=wo_sb[:, kk, do * 128 : (do + 1) * 128],
                                rhs=ocb[:, kk, :],
                                start=(kk == 0),
                                stop=(kk == HPC - 1),
                                skip_group_check=True,
                            )
                        ob = obpool.tile([128, TB], f16, name="ob", tag="ob")
                        # PSUM->SBUF copies: ACT during QKV phases (ACT is
                        # idle there, DVE runs the ropes); alternate DVE/ACT
                        # during attention (DVE gates the late blocks)
                        if spread and do % 2 == 1:
                            nc.scalar.copy(ob[:], pp[:])
                        elif spread:
                            nc.vector.tensor_copy(ob[:], pp[:])
                        elif ob_on_act[0]:
                            nc.scalar.copy(ob[:], pp[:])
                        else:
                            nc.vector.tensor_copy(ob[:], pp[:])
                        if spread:
                            # avoid the SWDGE queue at the tail: its
                            # transfers complete late and hold up teardown
                            qeng = nc.sync if do % 2 == 0 else nc.scalar
                        else:
                            qeng = nc.sync if do % 2 == 0 else nc.gpsimd
                        qeng.dma_start(
                            out[bb, do * 128 : (do + 1) * 128, tqp], ob[:]
                        )

                    return thunk

                for do in range(D // 128):
                    pending.append(mk(do))

            def drain_pending(k):
                for _ in range(min(k, len(pending))):
                    pending.pop(0)()

            for b in range(B):
                # attention helpers (defined before the QKV loop so the last
                # QKV block can pre-emit the first attention tiles)
                def s_mm(j4, h, i):
                    s = ps_tile("s_ps")
                    p = i - 4 * j4
                    # matmuls narrower than 256 free run at 1/4 rate, so
                    # pad the p=3 diagonal tile to 256 (extra cols are
                    # masked later)
                    c0 = min(128 * p, TB - 256) if p > 0 else 0
                    nc.tensor.matmul(
                        s[:, c0:],
                        lhsT=kT_sb[:, h, i * 128 : (i + 1) * 128],
                        rhs=qT_sb[:, h, j4 * TB + c0 : (j4 + 1) * TB],
                        start=True,
                        stop=True,
                        skip_group_check=True,
                    )
                    return s

                def exp_tile(j4, h, i, s):
                    e_sb = epool.tile([128, TB], f16, name="e_sb", tag="e")
                    p = i - 4 * j4
                    if p < 0:
                        nc.scalar.activation(e_sb[:], s[:], EXP, scale=SCALE)
                    else:
                        # diagonal tile: cols < 128p never read downstream
                        # (o/esum start at min(c0, TB-256)), the 128-wide
                        # band [128p, 128p+128) is triangular, cols >=
                        # 128p+128 fully valid
                        c0 = 128 * p
                        mc0 = min(c0, TB - 256)
                        nc.scalar.activation(
                            e_sb[:, c0:], s[:, c0:], EXP, scale=SCALE
                        )
                        nc.vector.tensor_mul(
                            e_sb[:, c0 : c0 + 128],
                            e_sb[:, c0 : c0 + 128],
                            mask_sb[:],
                        )
                        if mc0 < c0:
                            nc.vector.memset(e_sb[:, mc0:c0], 0)
                    return e_sb

                # fp16 patch chains: recompute q/k/v for tokens 0..127 at
                # full precision and overwrite the fp8-path values (fp8
                # noise doesn't wash out for early tokens' softmax). One
                # chain covers BOTH batches (free dim 256); batch 1 results
                # stash until after b1's main rope, then copy in.
                def emit_patch(w, m):
                    wf_sb = {"q": wqf_sb, "k": wkf_sb, "v": wvf_sb}[w]
                    pp = psp.tile([128, B * 128], f32, name="pfix", tag="ps")
                    for ko in range(KO):
                        nc.tensor.matmul(
                            pp[:],
                            lhsT=wf_sb[:, ko, m * 128 : (m + 1) * 128],
                            rhs=xfix_sb[:, ko, :],
                            start=(ko == 0),
                            stop=(ko == KO - 1),
                        )
                    if w == "v":
                        vtmp = rpool.tile([128, TB], f16, name="rtmp", tag="rtmp")
                        nc.scalar.activation(
                            vtmp[:, 0:256], pp[:], COPY, scale=1.0 / WS
                        )
                        vt_ps = psp.tile([128, 2, 128], f16, name="vt_ps", tag="ps")
                        for half in range(2):
                            nc.tensor.matmul(
                                vt_ps[:, half, :],
                                lhsT=vtmp[:, half * 128 : half * 128 + 128],
                                rhs=ident[:],
                                is_transpose=True,
                                start=(half == 0),
                                stop=(half == 1),
                                skip_group_check=True,
                            )
                        nc.scalar.copy(
                            vh_sb[:, 0, m * 128 : (m + 1) * 128], vt_ps[:, 0, :]
                        )
                        nc.scalar.copy(v_stash[:, m, :], vt_ps[:, 1, :])
                    else:
                        rr = rpool.tile([128, TB], f16, name="rtmp", tag="rtmp")
                        tmp = rpool.tile([128, TB], f16, name="rtmp", tag="rtmp")
                        nc.vector.tensor_mul(rr[:, 0:256], pp[:], cosfix[:])
                        nc.vector.tensor_mul(
                            tmp[0:64, 0:256], pp[64:128, :], sinfix[0:64, :]
                        )
                        nc.vector.tensor_mul(
                            tmp[64:128, 0:256], pp[0:64, :], sinfix[64:128, :]
                        )
                        nc.vector.tensor_add(rr[:, 0:256], rr[:, 0:256], tmp[:, 0:256])
                        dst = qT_sb if w == "q" else kT_sb
                        nc.vector.tensor_copy(dst[:, m, 0:128], rr[:, 0:128])
                        si = (0 if w == "q" else 2) + m
                        nc.vector.tensor_copy(qk_stash[:, si, :], rr[:, 128:256])

                def apply_stash(w, m):
                    if w == "v":
                        nc.scalar.copy(
                            vh_sb[:, 0, m * 128 : (m + 1) * 128], v_stash[:, m, :]
                        )
                    else:
                        si = (0 if w == "q" else 2) + m
                        dst = qT_sb if w == "q" else kT_sb
                        nc.vector.tensor_copy(dst[:, m, 0:128], qk_stash[:, si, :])

                patch_fn = emit_patch if b == 0 else apply_stash
                patch_queue = [
                    ("q", 0), ("q", 1), ("k", 0), ("k", 1), ("v", 0), ("v", 1)
                ]

                # carried across blocks: S psums / exp tiles pre-emitted at
                # the previous block's tail so the next block's PE/ACT work
                # is already queued while the divide chain drains
                s_pend = {}
                e_pend = {}
                # ============ QKV projections (+RoPE, v transpose) ============
                for nb in range(NTB):
                    tsl = slice(nb * TB, (nb + 1) * TB)
                    psums = {}

                    def v_par(m):
                        vtt = vtpool.tile([128, TB], f16, name="vtt", tag="vtt")
                        nc.scalar.activation(
                            vtt[:], psums["v", m][:], COPY, scale=1.0 / WS
                        )
                        vt_ps = psp.tile([128, 4, 128], f16, name="vt_ps", tag="ps")
                        for tti in range(4):
                            nc.tensor.matmul(
                                vt_ps[:, tti, :],
                                lhsT=vtt[:, tti * 128 : (tti + 1) * 128],
                                rhs=ident[:],
                                is_transpose=True,
                                start=(tti == 0),
                                stop=(tti == 3),
                                skip_group_check=True,
                            )
                        nc.scalar.copy(
                            vh_sb[:, nb * 4 : nb * 4 + 4, m * 128 : (m + 1) * 128],
                            vt_ps[:],
                        )

                    def rope_par(w, dst, m):
                        ps = psums[w, m]
                        tmp = rpool.tile([128, TB], f16, name="rtmp", tag="rtmp")
                        d = dst[:, m, tsl]
                        nc.vector.tensor_mul(d, ps[:], cos_sb[:, tsl])
                        nc.vector.tensor_mul(
                            tmp[0:64, :], ps[64:128, :], sin_sb[0:64, tsl]
                        )
                        nc.vector.tensor_mul(
                            tmp[64:128, :], ps[0:64, :], sin_sb[64:128, tsl]
                        )
                        nc.vector.tensor_add(d, d, tmp[:])

                    if b == 0 and nb == 0:
                        # first block: weights are still streaming in, so run
                        # the ko2-interleaved order that matches their arrival
                        for w in ("q", "k", "v"):
                            for m in range(HPC):
                                psums[w, m] = ps_tile(f"ps_{w}{m}")
                        xq = None
                        for ko2 in range(KO2):
                            if ko2 % 2 == 0:
                                q4 = ko2 // 2
                                xq = xt_pre.pop((b, nb, q4), None)
                                if xq is None:
                                    xq = load_xt(b, nb, q4)
                            rhs = xq[:, 2 * (ko2 % 2) : 2 * (ko2 % 2) + 2, :]
                            for w, w_sb in (
                                ("q", wq8_sb), ("k", wk8_sb), ("v", wv8_sb)
                            ):
                                for m in range(HPC):
                                    nc.tensor.matmul(
                                        psums[w, m][:],
                                        lhsT=w_sb[:, ko2, :, m * 128 : (m + 1) * 128],
                                        rhs=rhs,
                                        start=(ko2 == 0),
                                        stop=(ko2 == KO2 - 1),
                                        perf_mode=DR,
                                    )
                            if ko2 in (0, 1, 2):
                                load_w8_slice(ko2 + 3)
                            elif ko2 == 3:
                                load_w8_slice(6)
                                load_w8_slice(7)
                            if ko2 in (0, 2, 4):
                                # JIT the next x quad (needed at step ko2+2)
                                q4n = ko2 // 2 + 1
                                if (b, nb, q4n) not in xt_pre:
                                    xt_pre[b, nb, q4n] = load_xt(b, nb, q4n)
                            if ko2 == 4:
                                nc.gpsimd.dma_start(cos_sb[:], cosd)
                            elif ko2 == 5:
                                nc.gpsimd.dma_start(sin_sb[:], sind)
                            elif ko2 == 7:
                                # first fp16 patch weight; the rest (wkf, wvf,
                                # wo) are staggered over blocks 1-2 so their
                                # 4MB doesn't crowd out the x-quad stream
                                nc.scalar.dma_start(
                                    wqf_sb[:],
                                    wqfd.rearrange("(kk p) n -> p kk n", p=128),
                                )
                            if ko2 >= 4:
                                pk = ko2 - 4
                                xt_pre[b, nb + 1, pk] = load_xt(b, nb + 1, pk)
                        # q first: attention needs q of this block immediately
                        v_par(0)
                        rope_par("q", qT_sb, 0)
                        rope_par("q", qT_sb, 1)
                        v_par(1)
                        rope_par("k", kT_sb, 0)
                        rope_par("k", kT_sb, 1)
                    else:
                        # steady state: one output at a time over all 8 pair
                        # slices (x pairs prefetched during the previous
                        # block), post-processing each output while the next
                        # accumulates.
                        xts = []
                        for q4 in range(KO2 // 2):
                            xq = xt_pre.pop((b, nb, q4), None)
                            if xq is None:
                                xq = load_xt(b, nb, q4)
                            xts.append(xq)
                        if b == 0 and nb == 1:
                            # patch x (both batches) + tiled rope tables
                            nc.sync.dma_start(xfix_sb[:], xfixd)
                            for half in range(B):
                                hs = slice(half * 128, (half + 1) * 128)
                                nc.vector.tensor_copy(cosfix[:, hs], cos_sb[:, 0:128])
                                nc.vector.tensor_copy(sinfix[:, hs], sin_sb[:, 0:128])
                        if b == 0 and nb == 2:
                            nc.scalar.dma_start(
                                wvf_sb[:],
                                wvfd.rearrange("(kk p) n -> p kk n", p=128),
                            )

                        def mm_pass(w, m):
                            w_sb = {"q": wq8_sb, "k": wk8_sb, "v": wv8_sb}[w]
                            ps = ps_tile(f"ps_{w}{m}")
                            psums[w, m] = ps
                            for ko2 in range(KO2):
                                nc.tensor.matmul(
                                    ps[:],
                                    lhsT=w_sb[:, ko2, :, m * 128 : (m + 1) * 128],
                                    rhs=xts[ko2 // 2][
                                        :, 2 * (ko2 % 2) : 2 * (ko2 % 2) + 2, :
                                    ],
                                    start=(ko2 == 0),
                                    stop=(ko2 == KO2 - 1),
                                    perf_mode=DR,
                                )

                        def v_copy(m):
                            vtt = vtpool.tile([128, TB], f16, name="vtt", tag="vtt")
                            nc.scalar.activation(
                                vtt[:], psums["v", m][:], COPY, scale=1.0 / WS
                            )
                            return vtt

                        def v_tr(m, vtt):
                            # 4 transposes accumulate into ONE psum bank
                            # (start only on the first — a start marks the
                            # whole 2KB bank pending-zero), then a single
                            # strided ACT copy casts to fp8 vh slots
                            vt_ps = psp.tile([128, 4, 128], f16, name="vt_ps", tag="ps")
                            for tti in range(4):
                                nc.tensor.matmul(
                                    vt_ps[:, tti, :],
                                    lhsT=vtt[:, tti * 128 : (tti + 1) * 128],
                                    rhs=ident[:],
                                    is_transpose=True,
                                    start=(tti == 0),
                                    stop=(tti == 3),
                                    skip_group_check=True,
                                )
                            nc.scalar.copy(
                                vh_sb[:, nb * 4 : nb * 4 + 4, m * 128 : (m + 1) * 128],
                                vt_ps[:],
                            )

                        def pf(oi):
                            if nb + 1 < NTB and oi < 4:
                                if (b, nb + 1, oi) not in xt_pre:
                                    xt_pre[b, nb + 1, oi] = load_xt(b, nb + 1, oi)
                                # reach one block further for the first two
                                # quads: absorbs DMA bursts (weights/stores)
                                # without stalling the next block's matmuls
                                if oi < 2 and nb + 2 < NTB and (
                                    (b, nb + 2, oi) not in xt_pre
                                ):
                                    xt_pre[b, nb + 2, oi] = load_xt(b, nb + 2, oi)

                        def patch(k=1):
                            if nb >= 2:
                                for _ in range(min(k, len(patch_queue))):
                                    w, m = patch_queue.pop(0)
                                    patch_fn(w, m)

                        # each v pass only issues its ACT copy; the PE
                        # transposes run a pass later, once the copy has had
                        # a full matmul pass of latency cover. k1 goes last:
                        # its rope is DVE-only, so the block tail never
                        # blocks the PE.
                        ob_on_act[0] = True
                        mm_pass("q", 0)
                        if len(pending) > 4:
                            drain_pending(min(2, len(pending) - 4))
                        rope_par("q", qT_sb, 0)
                        pf(0)
                        patch()
                        mm_pass("q", 1)
                        if len(pending) > 4:
                            drain_pending(min(2, len(pending) - 4))
                        rope_par("q", qT_sb, 1)
                        pf(1)
                        patch()
                        mm_pass("k", 0)
                        if len(pending) > 4:
                            drain_pending(min(2, len(pending) - 4))
                        rope_par("k", kT_sb, 0)
                        pf(2)
                        patch()
                        mm_pass("v", 0)
                        vtt0 = v_copy(0)
                        if len(pending) > 4:
                            drain_pending(min(2, len(pending) - 4))
                        pf(3)
                        mm_pass("v", 1)
                        vtt1 = v_copy(1)
                        v_tr(0, vtt0)
                        if len(pending) > 4:
                            drain_pending(min(2, len(pending) - 4))
                        mm_pass("k", 1)
                        v_tr(1, vtt1)
                        rope_par("k", kT_sb, 1)
                        if len(pending) > 4:
                            drain_pending(min(2, len(pending) - 4))
                        if b == 0 and nb == 1:
                            nc.gpsimd.dma_start(
                                wkf_sb[:],
                                wkfd.rearrange("(kk p) n -> p kk n", p=128),
                            )
                        elif b == 0 and nb == 2:
                            nc.gpsimd.dma_start(
                                wo_sb[:],
                                woT.rearrange("(kk p) n -> p kk n", p=128),
                            )

                # any patches not yet emitted (shouldn't happen) run now
                while patch_queue:
                    w, m = patch_queue.pop(0)
                    patch_fn(w, m)

                # ============ attention (staggered heads) + spread proj ============
                ob_on_act[0] = False
                for j4 in range(NTB):
                    tq = slice(j4 * TB, (j4 + 1) * TB)
                    n_tk = 4 * (j4 + 1)
                    ocb = ocpool.tile([128, HPC, TB], f16, name="ocb", tag="ocb")
                    o_ps = [ps_tile(f"o_ps{h}") for h in range(HPC)]
                    # softmax denominators: E tiles summed on DVE (fp16),
                    # finished by one small ones-matmul per head — keeps
                    # ~30us of denominator matmuls off the PE
                    esum = [
                        espool.tile([128, TB], f16, name=f"esum{h}", tag="es")
                        for h in range(HPC)
                    ]

                    def o_den_mm(h, i, e_sb):
                        p = i - 4 * j4
                        c0 = min(128 * p, TB - 256) if p > 0 else 0
                        nc.tensor.matmul(
                            o_ps[h][:, c0:],
                            lhsT=vh_sb[:, i, h * 128 : (h + 1) * 128],
                            rhs=e_sb[:, c0:],
                            start=(i == 0),
                            stop=(i == n_tk - 1),
                            skip_group_check=True,
                        )
                        if i == 0:
                            nc.vector.tensor_copy(esum[h][:], e_sb[:])
                        else:
                            nc.vector.tensor_add(
                                esum[h][:, c0:], esum[h][:, c0:], e_sb[:, c0:]
                            )

                    def emit_div(h):
                        den = ps_tile("den")
                        nc.tensor.matmul(
                            den[:],
                            lhsT=ones_sb[:],
                            rhs=esum[h][:],
                            start=True,
                            stop=True,
                            skip_group_check=True,
                        )
                        lnd = rcpool.tile([128, TB], f32, name="lnd", tag="lnd")
                        nc.scalar.activation(
                            lnd[:], den[:], mybir.ActivationFunctionType.Ln
                        )
                        recip = rcpool.tile([128, TB], f32, name="recip", tag="rcp")
                        nc.scalar.activation(recip[:], lnd[:], EXP, scale=-1.0)
                        nc.vector.tensor_mul(ocb[:, h, :], o_ps[h][:], recip[:])

                    if j4 == 0:
                        # all of head 0 first: every h1 S tile here needs
                        # this batch's k1, whose rope is the QKV-phase tail
                        # on DVE — h0's full pass covers that latency
                        for h in range(HPC):
                            if (h, 0) not in s_pend and (h, 0) not in e_pend:
                                s_pend[h, 0] = s_mm(j4, h, 0)
                            for i in range(n_tk):
                                if i + 1 < n_tk and (h, i + 1) not in s_pend:
                                    s_pend[h, i + 1] = s_mm(j4, h, i + 1)
                                elif i + 1 == n_tk and h + 1 < HPC:
                                    s_pend[h + 1, 0] = s_mm(j4, h + 1, 0)
                                e = e_pend.pop((h, i), None)
                                if e is None:
                                    e = exp_tile(j4, h, i, s_pend.pop((h, i)))
                                o_den_mm(h, i, e)
                            if h == 0:
                                emit_div(0)
                            else:
                                ns00 = s_mm(j4 + 1, 0, 0)
                                s_pend[1, 0] = s_mm(j4 + 1, 1, 0)
                                s_pend[0, 1] = s_mm(j4 + 1, 0, 1)
                                e_pend[0, 0] = exp_tile(j4 + 1, 0, 0, ns00)
                            drain_pending(2)
                        emit_div(1)
                    else:
                        for i in range(n_tk):
                            if (1, i) not in s_pend:
                                s_pend[1, i] = s_mm(j4, 1, i)
                            if i + 1 < n_tk and (0, i + 1) not in s_pend:
                                s_pend[0, i + 1] = s_mm(j4, 0, i + 1)
                            e0 = e_pend.pop((0, i), None)
                            if e0 is None:
                                e0 = exp_tile(j4, 0, i, s_pend.pop((0, i)))
                            o_den_mm(0, i, e0)
                            if i == n_tk - 1:
                                # head 0 finished: divide now so its o/den
                                # psum banks free before the next block
                                emit_div(0)
                            e1 = exp_tile(j4, 1, i, s_pend.pop((1, i)))
                            o_den_mm(1, i, e1)
                            if i == n_tk - 1 and j4 + 1 < NTB:
                                # pre-emit the next block's first S matmuls
                                # and exp ahead of this block's divide chain,
                                # so neither the PE nor ACT queue drains dry
                                # at the block boundary
                                ns00 = s_mm(j4 + 1, 0, 0)
                                s_pend[1, 0] = s_mm(j4 + 1, 1, 0)
                                s_pend[0, 1] = s_mm(j4 + 1, 0, 1)
                                e_pend[0, 0] = exp_tile(j4 + 1, 0, 0, ns00)
                            # drain the out-projection backlog, keeping >=4
                            # thunks in reserve for the boundary
                            if 1 <= i < n_tk - 2 and len(pending) > 4:
                                drain_pending(min(3, len(pending) - 4))
                        emit_div(1)
                    drain_pending(4)
                    emit_proj_block(
                        b, j4, ocb, spread=(b == B - 1 and j4 == NTB - 1)
                    )
                if b + 1 < B:
                    # prefetch the next batch's whole first x block across
                    # the QKV-phase boundary
                    for q4 in range(KO2 // 2):
                        xt_pre[b + 1, 0, q4] = load_xt(b + 1, 0, q4)
            drain_pending(len(pending))
    return nc


def prepare_inputs(x, rope_freqs, w_q, w_k, w_v, w_o):
    """Host-side sharding/layout prep. Returns per-core input maps."""
    import ml_dtypes

    E4 = ml_dtypes.float8_e4m3

    x = np.asarray(x, dtype=np.float32)
    rope_freqs = np.asarray(rope_freqs, dtype=np.float32)
    w_q = np.asarray(w_q, dtype=np.float32)
    w_k = np.asarray(w_k, dtype=np.float32)
    w_v = np.asarray(w_v, dtype=np.float32)
    w_o = np.asarray(w_o, dtype=np.float32)

    xT = np.ascontiguousarray(x.transpose(0, 2, 1))  # [B, D, T] fp32
    x8T = xT.astype(E4)
    # patch x: tokens 0..127 of BOTH batches, laid out [128p, ko, b*128+t]
    xfix = np.ascontiguousarray(
        xT[:, :, 0:128].reshape(B, KO, 128, 128).transpose(2, 1, 0, 3)
        .reshape(128, KO, B * 128)
        .astype(np.float16)
    )

    # permute q/k weight rows within each head: even HD idx -> rows 0..63,
    # odd -> rows 64..127 (so RoPE pairing becomes a half swap)
    perm = np.concatenate([np.arange(0, HD, 2), np.arange(1, HD, 2)])
    rows = (np.arange(D).reshape(H, HD)[:, perm]).reshape(D)
    w_qp = w_q[rows] * WS
    w_kp = w_k[rows] * WS
    w_vs = w_v * WS

    cos = rope_freqs[..., 0].T / WS  # [64, T]
    sin = rope_freqs[..., 1].T / WS
    cos_sb = np.ascontiguousarray(np.concatenate([cos, cos], axis=0))  # [128, T]
    sin_sb = np.ascontiguousarray(np.concatenate([-sin, sin], axis=0))

    def pack8(wT):  # wT [D, CD] fp32 -> [128, KO2, 2, CD] e4m3
        return np.ascontiguousarray(
            wT.reshape(KO2, 2, 128, CD).transpose(2, 0, 1, 3).astype(E4)
        )

    in_maps = []
    for cidx in range(NCORES):
        sl = slice(cidx * CD, (cidx + 1) * CD)
        wqT = np.ascontiguousarray(w_qp[sl].T)  # [D, CD] fp32, x WS
        wkT = np.ascontiguousarray(w_kp[sl].T)
        wvT = np.ascontiguousarray(w_vs[sl].T)
        in_maps.append(
            {
                "x8T": x8T,
                "xfixd": xfix,
                "wq8d": pack8(wqT),
                "wk8d": pack8(wkT),
                "wv8d": pack8(wvT),
                "wqfd": wqT.astype(np.float16),
                "wkfd": wkT.astype(np.float16),
                "wvfd": wvT.astype(np.float16),
                "woT": np.ascontiguousarray(w_o[:, sl].T.astype(np.float16)),
                "cosd": cos_sb.astype(np.float16),
                "sind": sin_sb.astype(np.float16),
            }
        )
    return in_maps


def run(in_maps, trace=False, tmpdir=None):
    from concourse.bass_utils import run_bass_kernel_spmd

    nc = build_bass()
    res = run_bass_kernel_spmd(
        nc,
        in_maps,
        core_ids=list(range(NCORES)),
        trace=trace,
        tmpdir=tmpdir,
    )
    total = np.zeros((B, D, T), dtype=np.float32)
    for cres in res.results:
        total += cres["out"].astype(np.float32)
    final = np.ascontiguousarray(total.transpose(0, 2, 1))  # [B, T, D]
    return final, res


def kernel(x, rope_freqs, w_q, w_k, w_v, w_o):
    in_maps = prepare_inputs(x, rope_freqs, w_q, w_k, w_v, w_o)
    final, _ = run(in_maps, trace=False)
    return final
